# revision 5
# baseline (speedup 1.0000x reference)
"""MultiHeadAttention TRN2 Bass kernel (nn_MultiHeadAttention_51565377356100).

Full inputs in, full outputs out. Sharding: 8 cores = 2 batches x 4 head-groups
(16 of the 64 effective heads each). Each core computes, for its (batch b,
heads hg*16..hg*16+15):
  qhT/khT = (x @ W{q,k} + b)^T per head      [fp16, via f32r-free fp16 matmuls]
  vh'     = x @ Wv + b, with a ones-column appended (rowsum rides attn@V)
  S       = qh @ kh^T (raw dot, scale 1/8 fused into exp)
  attnU   = exp(S/8)  [ACT, fp16, fp32 accum_out -> rowsum]
  attn_acc += attnU * (1/rowsum)   [fused scalar_tensor_tensor, fp16]
  attnUT  = DMA-xbar transpose of attnU (fp16)
  O'      = [vh | 1]^T @ attnUT    -> row 64 = fp16-consistent rowsum
  OT      = O'[0:64] * broadcast(1/rowsum)   [K=1 ones matmul + DVE mul]
  out_part = sum_h OT_h^T @ Wf_h   [PSUM-accumulated over the 16 heads]
Host: out = sum over 4 head-group cores + bf;  attn_sum likewise (then the
mask is all-False by construction, so it is a no-op).
"""
import numpy as np

import concourse.bass as bass
import concourse.tile as tile
from concourse import bacc, mybir
from concourse.bass_utils import run_bass_kernel_spmd

F32 = mybir.dt.float32
F32R = mybir.dt.float32r
F16 = mybir.dt.float16
AF = mybir.ActivationFunctionType
ALU = mybir.AluOpType

B, S, D = 2, 1024, 512
NH = 16          # heads per core (of 64 total)
SZ = 64          # head dim
DP = NH * SZ     # per-core projection width = 1024
N_CORES = 8

_NC_CACHE = None


def build_nc():
    nc = bacc.Bacc("TRN2", target_bir_lowering=False, debug=False,
                   num_devices=N_CORES)

    # ---- DRAM I/O (per-core shapes) ----
    xqT = nc.dram_tensor("xqT", [D, S], F16, kind="ExternalInput").ap()
    xkT = nc.dram_tensor("xkT", [D, S], F16, kind="ExternalInput").ap()
    xvT = nc.dram_tensor("xvT", [D, S], F16, kind="ExternalInput").ap()
    wq = nc.dram_tensor("wq", [D, DP], F16, kind="ExternalInput").ap()
    wk = nc.dram_tensor("wk", [D, DP], F16, kind="ExternalInput").ap()
    wv = nc.dram_tensor("wv", [D, DP], F16, kind="ExternalInput").ap()
    wf = nc.dram_tensor("wf", [NH, SZ, D], F16, kind="ExternalInput").ap()
    bq = nc.dram_tensor("bq", [DP], F32, kind="ExternalInput").ap()
    bk = nc.dram_tensor("bk", [DP], F32, kind="ExternalInput").ap()
    bv = nc.dram_tensor("bv", [1, DP], F32R, kind="ExternalInput").ap()

    out_part = nc.dram_tensor("out_part", [S, D], F32, kind="ExternalOutput").ap()
    attn_part = nc.dram_tensor("attn_part", [S, S], F16, kind="ExternalOutput").ap()

    KT = D // 128            # 4 contraction tiles for projections
    MT = DP // 128           # 8 outdim tiles (2 heads each)
    QT = S // 128            # 8 token tiles
    KB = S // 128            # 8 key tiles

    with tile.TileContext(nc) as tc:
        with (
            tc.tile_pool(name="fix", bufs=1) as fix,          # persistent tensors
            tc.tile_pool(name="stream", bufs=3) as stream,    # attnU tiles
            tc.tile_pool(name="tp", bufs=2) as tp,            # attnUT double-buffer
            tc.tile_pool(name="sm", bufs=2) as sm,            # small stats tiles
            tc.tile_pool(name="stage", bufs=2) as stage,      # psum->sbuf staging
            tc.tile_pool(name="ps1", bufs=2, space="PSUM") as ps1,   # proj/fc psum
            tc.tile_pool(name="pso", bufs=1, space="PSUM") as pso,   # O' psum
            tc.tile_pool(name="psr", bufs=1, space="PSUM") as psr,   # rb psum
            tc.tile_pool(name="ps2", bufs=2, space="PSUM") as ps2,   # scores psum
        ):
            # ---------- load inputs ----------
            xq_sb = tp.tile([128, KT, S], F16, tag="attnUT")
            xk_sb = tp.tile([128, KT, S], F16, tag="attnUT")
            xv_sb = tp.tile([128, KT, S], F16, tag="attnUT")
            for kt in range(KT):
                nc.sync.dma_start(out=xq_sb[:, kt, :], in_=xqT[kt * 128:(kt + 1) * 128, :])
                nc.sync.dma_start(out=xk_sb[:, kt, :], in_=xkT[kt * 128:(kt + 1) * 128, :])
                nc.sync.dma_start(out=xv_sb[:, kt, :], in_=xvT[kt * 128:(kt + 1) * 128, :])
            wq_sb = stage.tile([128, KT, DP], F16, tag="W")
            wk_sb = stage.tile([128, KT, DP], F16, tag="W")
            wv_sb = stage.tile([128, KT, DP], F16, tag="W")
            for kt in range(KT):
                nc.sync.dma_start(out=wq_sb[:, kt, :], in_=wq[kt * 128:(kt + 1) * 128, :])
                nc.sync.dma_start(out=wk_sb[:, kt, :], in_=wk[kt * 128:(kt + 1) * 128, :])
                nc.sync.dma_start(out=wv_sb[:, kt, :], in_=wv[kt * 128:(kt + 1) * 128, :])
            wf_sb = fix.tile([64, NH, D], F16, tag="wf")
            for h in range(NH):
                nc.sync.dma_start(out=wf_sb[:, h, :], in_=wf[h, :, :])
            bq_sb = fix.tile([128, MT], F32, tag="bq")
            bk_sb = fix.tile([128, MT], F32, tag="bk")
            nc.sync.dma_start(out=bq_sb, in_=bq.rearrange("(m p) -> p m", p=128))
            nc.sync.dma_start(out=bk_sb, in_=bk.rearrange("(m p) -> p m", p=128))
            bv_row = fix.tile([1, DP], F32R, tag="bvrow")
            nc.sync.dma_start(out=bv_row, in_=bv)

            # ones rows (f32r) for K=1 broadcast matmuls; row 64 used for rb.
            ones_f = fix.tile([128, 128], F32, tag="onesf")
            nc.vector.memset(ones_f, 1.0)
            ones_sb = fix.tile([128, 128], F32R, tag="ones")
            nc.vector.tensor_copy(ones_sb, ones_f)

            # bv broadcast [128, DP] fp32 via K=1 matmul
            bv_bc = fix.tile([128, DP], F16, tag="bvbc")
            for n5 in range(DP // 512):
                pbc = ps1.tile([128, 512], F32, tag="p1")
                nc.tensor.matmul(pbc, ones_sb[0:1, :], bv_row[:, n5 * 512:(n5 + 1) * 512],
                                 start=True, stop=True)
                nc.vector.tensor_copy(bv_bc[:, n5 * 512:(n5 + 1) * 512], pbc)

            # ---------- projections ----------
            qhT_sb = fix.tile([128, MT, S], F16, tag="qhT")
            khT_sb = fix.tile([128, MT, S], F16, tag="khT")
            for (w_sb, x_sb, b_sb, o_sb) in ((wq_sb, xq_sb, bq_sb, qhT_sb),
                                             (wk_sb, xk_sb, bk_sb, khT_sb)):
                for m in range(MT):
                    for n in range(S // 512):
                        pp = ps1.tile([128, 512], F32, tag="p1")
                        for kt in range(KT):
                            nc.tensor.matmul(
                                pp,
                                w_sb[:, kt, m * 128:(m + 1) * 128],
                                x_sb[:, kt, n * 512:(n + 1) * 512],
                                start=(kt == 0), stop=(kt == KT - 1))
                        nc.scalar.activation(
                            out=o_sb[:, m, n * 512:(n + 1) * 512], in_=pp,
                            func=AF.Identity, bias=b_sb[:, m:m + 1], scale=1.0)

            # vh' [128, tt, NH, 65] fp16; col 64 = 1.0
            vhp = fix.tile([128, QT, NH, SZ + 1], F16, tag="vhp")
            nc.vector.memset(vhp[:, :, :, SZ:SZ + 1], 1.0)
            for tt in range(QT):
                for n5 in range(DP // 512):
                    pv = ps1.tile([128, 512], F32, tag="p1")
                    for kt in range(KT):
                        nc.tensor.matmul(
                            pv,
                            xv_sb[:, kt, tt * 128:(tt + 1) * 128],
                            wv_sb[:, kt, n5 * 512:(n5 + 1) * 512],
                            start=(kt == 0), stop=(kt == KT - 1))
                    nc.vector.tensor_add(
                        vhp[:, tt, n5 * 8:(n5 + 1) * 8, 0:SZ],
                        pv.rearrange("p (h d) -> p h d", d=SZ),
                        bv_bc[:, n5 * 512:(n5 + 1) * 512].rearrange(
                            "p (h d) -> p h d", d=SZ))

            # ---------- attention ----------
            attn_acc = fix.tile([128, QT, S], F16, tag="acc")
            out_acc_written = False
            OT = fix.tile([64, NH, S], F16, tag="OT")
            for h in range(NH):
                m, po = h // 2, (h % 2) * 64
                attnUT = tp.tile([128, KB, S], F16, tag="attnUT")
                for qt in range(QT):
                    sp = ps2.tile([128, S], F32, tag="p2")
                    for k5 in range(S // 512):
                        nc.tensor.matmul(
                            sp[:, k5 * 512:(k5 + 1) * 512],
                            qhT_sb[po:po + 64, m, qt * 128:(qt + 1) * 128],
                            khT_sb[po:po + 64, m, k5 * 512:(k5 + 1) * 512],
                            start=True, stop=True)
                    attnU = stream.tile([128, S], F16, tag="attnU")
                    rs = sm.tile([128, 1], F32, tag="rs")
                    nc.scalar.activation(out=attnU, in_=sp, func=AF.Exp,
                                         scale=0.125, accum_out=rs)
                    r = sm.tile([128, 1], F32, tag="r")
                    nc.vector.reciprocal(r, rs)
                    if h == 0:
                        nc.vector.tensor_scalar_mul(attn_acc[:, qt, :], attnU, r)
                    else:
                        nc.vector.scalar_tensor_tensor(
                            out=attn_acc[:, qt, :], in0=attnU, scalar=r,
                            in1=attn_acc[:, qt, :], op0=ALU.mult, op1=ALU.add)
                    nc.sync.dma_start_transpose(
                        out=attnUT[:, :, qt * 128:(qt + 1) * 128], in_=attnU)
                for q5 in range(S // 512):
                    po_ = pso.tile([65, 512], F32, tag="po")
                    for kb in range(KB):
                        nc.tensor.matmul(
                            po_,
                            vhp[:, kb, h, :],
                            attnUT[:, kb, q5 * 512:(q5 + 1) * 512],
                            start=(kb == 0), stop=(kb == KB - 1))
                    rso = sm.tile([65, 512], F32R, tag="rso")
                    with nc.allow_low_precision(reason="f32r recip feeding f32r matmul"):
                        nc.vector.reciprocal(rso[64:65, :], po_[64:65, :])
                    rbp = psr.tile([64, 512], F32, tag="rb")
                    nc.tensor.matmul(rbp, ones_sb[64:65, 0:64], rso[64:65, :],
                                     start=True, stop=True)
                    rb_sb = stage.tile([64, 512], F32, tag="rbsb")
                    nc.vector.tensor_copy(rb_sb, rbp)
                    nc.vector.tensor_mul(OT[:, h, q5 * 512:(q5 + 1) * 512],
                                         po_[0:64, :], rb_sb)

            # ---------- fc + outputs ----------
            for qt in range(QT):
                pf = ps1.tile([128, 512], F32, tag="p1")
                for h in range(NH):
                    nc.tensor.matmul(
                        pf,
                        OT[:, h, qt * 128:(qt + 1) * 128],
                        wf_sb[:, h, :],
                        start=(h == 0), stop=(h == NH - 1))
                o_sb = stage.tile([128, 512], F32, tag="osb")
                nc.scalar.copy(o_sb, pf)
                nc.sync.dma_start(out=out_part[qt * 128:(qt + 1) * 128, :], in_=o_sb)
                nc.sync.dma_start(out=attn_part[qt * 128:(qt + 1) * 128, :],
                                  in_=attn_acc[:, qt, :])

    nc.compile()
    return nc


def _get_nc():
    global _NC_CACHE
    if _NC_CACHE is None:
        _NC_CACHE = build_nc()
    return _NC_CACHE


def kernel(q, k, v, mask, Wq, bq, Wk, bk, Wv, bv, Wf, bf):
    q = np.asarray(q); k = np.asarray(k); v = np.asarray(v)
    Wq = np.asarray(Wq); Wk = np.asarray(Wk); Wv = np.asarray(Wv)
    Wf = np.asarray(Wf)
    bq = np.asarray(bq, dtype=np.float32)
    bk = np.asarray(bk, dtype=np.float32)
    bv = np.asarray(bv, dtype=np.float32)
    bf = np.asarray(bf, dtype=np.float32)

    nc = _get_nc()
    in_maps = []
    for c in range(N_CORES):
        b, g = c // 4, c % 4
        cs = slice(g * DP, (g + 1) * DP)          # projection columns
        in_maps.append({
            "xqT": np.ascontiguousarray(q[b].T.astype(np.float16)),
            "xkT": np.ascontiguousarray(k[b].T.astype(np.float16)),
            "xvT": np.ascontiguousarray(v[b].T.astype(np.float16)),
            "wq": np.ascontiguousarray(Wq[:, cs].astype(np.float16)),
            "wk": np.ascontiguousarray(Wk[:, cs].astype(np.float16)),
            "wv": np.ascontiguousarray(Wv[:, cs].astype(np.float16)),
            "wf": np.ascontiguousarray(
                Wf[cs, :].astype(np.float16).reshape(NH, SZ, D)),
            "bq": np.ascontiguousarray(bq[cs]),
            "bk": np.ascontiguousarray(bk[cs]),
            "bv": np.ascontiguousarray(bv[cs].reshape(1, DP)),
        })
    res = run_bass_kernel_spmd(nc, in_maps, list(range(N_CORES)))

    out = np.zeros((B, S, D), np.float32)
    attn_sum = np.zeros((B, S, S), np.float32)
    for c in range(N_CORES):
        b = c // 4
        out[b] += res.results[c]["out_part"]
        attn_sum[b] += res.results[c]["attn_part"].astype(np.float32)
    out += bf[None, None, :]
    return out, attn_sum


# revision 7
# speedup vs baseline: 811.0015x; 811.0015x over previous
"""MultiHeadAttention TRN2 Bass kernel (nn_MultiHeadAttention_51565377356100).

Full inputs in, full outputs out. Sharding: 8 cores = 2 batches x 4 head-groups
(16 of the 64 effective heads each). Each core computes, for its (batch b,
heads hg*16..hg*16+15):
  qhT/khT = (x @ W{q,k} + b)^T per head      [fp16, via f32r-free fp16 matmuls]
  vh'     = x @ Wv + b, with a ones-column appended (rowsum rides attn@V)
  S       = qh @ kh^T (raw dot, scale 1/8 fused into exp)
  attnU   = exp(S/8)  [ACT, fp16, fp32 accum_out -> rowsum]
  attn_acc += attnU * (1/rowsum)   [fused scalar_tensor_tensor, fp16]
  attnUT  = DMA-xbar transpose of attnU (fp16)
  O'      = [vh | 1]^T @ attnUT    -> row 64 = fp16-consistent rowsum
  OT      = O'[0:64] * broadcast(1/rowsum)   [K=1 ones matmul + DVE mul]
  out_part = sum_h OT_h^T @ Wf_h   [PSUM-accumulated over the 16 heads]
Host: out = sum over 4 head-group cores + bf;  attn_sum likewise (then the
mask is all-False by construction, so it is a no-op).
"""
import numpy as np

import concourse.bass as bass
import concourse.tile as tile
from concourse import bacc, mybir
from concourse.bass_utils import run_bass_kernel_spmd

F32 = mybir.dt.float32
F32R = mybir.dt.float32r
F16 = mybir.dt.float16
AF = mybir.ActivationFunctionType
ALU = mybir.AluOpType

B, S, D = 2, 1024, 512
NH = 16          # heads per core (of 64 total)
SZ = 64          # head dim
DP = NH * SZ     # per-core projection width = 1024
N_CORES = 8

_NC_CACHE = None
import contextlib

def _nullcm():
    return contextlib.nullcontext()


def build_nc(loop_n=1):
    nc = bacc.Bacc("TRN2", target_bir_lowering=False, debug=False,
                   num_devices=N_CORES)

    # ---- DRAM I/O (per-core shapes) ----
    xqT = nc.dram_tensor("xqT", [D, S], F16, kind="ExternalInput").ap()
    xkT = nc.dram_tensor("xkT", [D, S], F16, kind="ExternalInput").ap()
    xvT = nc.dram_tensor("xvT", [D, S], F16, kind="ExternalInput").ap()
    wq = nc.dram_tensor("wq", [D, DP], F16, kind="ExternalInput").ap()
    wk = nc.dram_tensor("wk", [D, DP], F16, kind="ExternalInput").ap()
    wv = nc.dram_tensor("wv", [D, DP], F16, kind="ExternalInput").ap()
    wf = nc.dram_tensor("wf", [NH, SZ, D], F16, kind="ExternalInput").ap()
    bq = nc.dram_tensor("bq", [DP], F32, kind="ExternalInput").ap()
    bk = nc.dram_tensor("bk", [DP], F32, kind="ExternalInput").ap()
    bv = nc.dram_tensor("bv", [1, DP], F32R, kind="ExternalInput").ap()

    out_part = nc.dram_tensor("out_part", [S, D], F32, kind="ExternalOutput").ap()
    attn_part = nc.dram_tensor("attn_part", [S, S], F16, kind="ExternalOutput").ap()

    KT = D // 128            # 4 contraction tiles for projections
    MT = DP // 128           # 8 outdim tiles (2 heads each)
    QT = S // 128            # 8 token tiles
    KB = S // 128            # 8 key tiles

    with tile.TileContext(nc) as tc:
        with (
            tc.tile_pool(name="fix", bufs=1) as fix,          # persistent tensors
            tc.tile_pool(name="stream", bufs=3) as stream,    # attnU tiles
            tc.tile_pool(name="tp", bufs=2) as tp,            # attnUT double-buffer
            tc.tile_pool(name="sm", bufs=2) as sm,            # small stats tiles
            tc.tile_pool(name="stage", bufs=2) as stage,      # psum->sbuf staging
            tc.tile_pool(name="ps1", bufs=2, space="PSUM") as ps1,   # proj/fc psum
            tc.tile_pool(name="pso", bufs=1, space="PSUM") as pso,   # O' psum
            tc.tile_pool(name="psr", bufs=1, space="PSUM") as psr,   # rb psum
            tc.tile_pool(name="ps2", bufs=2, space="PSUM") as ps2,   # scores psum
        ):
          with (tc.For_i(0, loop_n, 1) if loop_n > 1 else _nullcm()):
            # ---------- load inputs ----------
            xq_sb = tp.tile([128, KT, S], F16, tag="attnUT")
            xk_sb = tp.tile([128, KT, S], F16, tag="attnUT")
            xv_sb = tp.tile([128, KT, S], F16, tag="attnUT")
            for kt in range(KT):
                nc.sync.dma_start(out=xq_sb[:, kt, :], in_=xqT[kt * 128:(kt + 1) * 128, :])
                nc.sync.dma_start(out=xk_sb[:, kt, :], in_=xkT[kt * 128:(kt + 1) * 128, :])
                nc.sync.dma_start(out=xv_sb[:, kt, :], in_=xvT[kt * 128:(kt + 1) * 128, :])
            wq_sb = stage.tile([128, KT, DP], F16, tag="W")
            wk_sb = stage.tile([128, KT, DP], F16, tag="W")
            wv_sb = stage.tile([128, KT, DP], F16, tag="W")
            for kt in range(KT):
                nc.sync.dma_start(out=wq_sb[:, kt, :], in_=wq[kt * 128:(kt + 1) * 128, :])
                nc.sync.dma_start(out=wk_sb[:, kt, :], in_=wk[kt * 128:(kt + 1) * 128, :])
                nc.sync.dma_start(out=wv_sb[:, kt, :], in_=wv[kt * 128:(kt + 1) * 128, :])
            wf_sb = fix.tile([64, NH, D], F16, tag="wf")
            for h in range(NH):
                nc.sync.dma_start(out=wf_sb[:, h, :], in_=wf[h, :, :])
            bq_sb = fix.tile([128, MT], F32, tag="bq")
            bk_sb = fix.tile([128, MT], F32, tag="bk")
            nc.sync.dma_start(out=bq_sb, in_=bq.rearrange("(m p) -> p m", p=128))
            nc.sync.dma_start(out=bk_sb, in_=bk.rearrange("(m p) -> p m", p=128))
            bv_row = fix.tile([1, DP], F32R, tag="bvrow")
            nc.sync.dma_start(out=bv_row, in_=bv)

            # ones rows (f32r) for K=1 broadcast matmuls; row 64 used for rb.
            ones_f = fix.tile([128, 128], F32, tag="onesf")
            nc.vector.memset(ones_f, 1.0)
            ones_sb = fix.tile([128, 128], F32R, tag="ones")
            nc.vector.tensor_copy(ones_sb, ones_f)

            # bv broadcast [128, DP] fp32 via K=1 matmul
            bv_bc = fix.tile([128, DP], F16, tag="bvbc")
            for n5 in range(DP // 512):
                pbc = ps1.tile([128, 512], F32, tag="p1")
                nc.tensor.matmul(pbc, ones_sb[0:1, :], bv_row[:, n5 * 512:(n5 + 1) * 512],
                                 start=True, stop=True)
                nc.vector.tensor_copy(bv_bc[:, n5 * 512:(n5 + 1) * 512], pbc)

            # ---------- projections ----------
            qhT_sb = fix.tile([128, MT, S], F16, tag="qhT")
            khT_sb = fix.tile([128, MT, S], F16, tag="khT")
            for (w_sb, x_sb, b_sb, o_sb) in ((wq_sb, xq_sb, bq_sb, qhT_sb),
                                             (wk_sb, xk_sb, bk_sb, khT_sb)):
                for m in range(MT):
                    for n in range(S // 512):
                        pp = ps1.tile([128, 512], F32, tag="p1")
                        for kt in range(KT):
                            nc.tensor.matmul(
                                pp,
                                w_sb[:, kt, m * 128:(m + 1) * 128],
                                x_sb[:, kt, n * 512:(n + 1) * 512],
                                start=(kt == 0), stop=(kt == KT - 1))
                        nc.scalar.activation(
                            out=o_sb[:, m, n * 512:(n + 1) * 512], in_=pp,
                            func=AF.Identity, bias=b_sb[:, m:m + 1], scale=1.0)

            # vh' [128, tt, NH, 65] fp16; col 64 = 1.0
            vhp = fix.tile([128, QT, NH, SZ + 1], F16, tag="vhp")
            nc.vector.memset(vhp[:, :, :, SZ:SZ + 1], 1.0)
            for tt in range(QT):
                for n5 in range(DP // 512):
                    pv = ps1.tile([128, 512], F32, tag="p1")
                    for kt in range(KT):
                        nc.tensor.matmul(
                            pv,
                            xv_sb[:, kt, tt * 128:(tt + 1) * 128],
                            wv_sb[:, kt, n5 * 512:(n5 + 1) * 512],
                            start=(kt == 0), stop=(kt == KT - 1))
                    nc.vector.tensor_add(
                        vhp[:, tt, n5 * 8:(n5 + 1) * 8, 0:SZ],
                        pv.rearrange("p (h d) -> p h d", d=SZ),
                        bv_bc[:, n5 * 512:(n5 + 1) * 512].rearrange(
                            "p (h d) -> p h d", d=SZ))

            # ---------- attention ----------
            attn_acc = fix.tile([128, QT, S], F16, tag="acc")
            out_acc_written = False
            OT = fix.tile([64, NH, S], F16, tag="OT")
            for h in range(NH):
                m, po = h // 2, (h % 2) * 64
                attnUT = tp.tile([128, KB, S], F16, tag="attnUT")
                for qt in range(QT):
                    sp = ps2.tile([128, S], F32, tag="p2")
                    for k5 in range(S // 512):
                        nc.tensor.matmul(
                            sp[:, k5 * 512:(k5 + 1) * 512],
                            qhT_sb[po:po + 64, m, qt * 128:(qt + 1) * 128],
                            khT_sb[po:po + 64, m, k5 * 512:(k5 + 1) * 512],
                            start=True, stop=True)
                    attnU = stream.tile([128, S], F16, tag="attnU")
                    rs = sm.tile([128, 1], F32, tag="rs")
                    nc.scalar.activation(out=attnU, in_=sp, func=AF.Exp,
                                         scale=0.125, accum_out=rs)
                    r = sm.tile([128, 1], F32, tag="r")
                    nc.vector.reciprocal(r, rs)
                    if h == 0:
                        nc.vector.tensor_scalar_mul(attn_acc[:, qt, :], attnU, r)
                    else:
                        nc.vector.scalar_tensor_tensor(
                            out=attn_acc[:, qt, :], in0=attnU, scalar=r,
                            in1=attn_acc[:, qt, :], op0=ALU.mult, op1=ALU.add)
                    nc.sync.dma_start_transpose(
                        out=attnUT[:, :, qt * 128:(qt + 1) * 128], in_=attnU)
                for q5 in range(S // 512):
                    po_ = pso.tile([65, 512], F32, tag="po")
                    for kb in range(KB):
                        nc.tensor.matmul(
                            po_,
                            vhp[:, kb, h, :],
                            attnUT[:, kb, q5 * 512:(q5 + 1) * 512],
                            start=(kb == 0), stop=(kb == KB - 1))
                    rso = sm.tile([65, 512], F32R, tag="rso")
                    with nc.allow_low_precision(reason="f32r recip feeding f32r matmul"):
                        nc.vector.reciprocal(rso[64:65, :], po_[64:65, :])
                    rbp = psr.tile([64, 512], F32, tag="rb")
                    nc.tensor.matmul(rbp, ones_sb[64:65, 0:64], rso[64:65, :],
                                     start=True, stop=True)
                    rb_sb = stage.tile([64, 512], F32, tag="rbsb")
                    nc.vector.tensor_copy(rb_sb, rbp)
                    nc.vector.tensor_mul(OT[:, h, q5 * 512:(q5 + 1) * 512],
                                         po_[0:64, :], rb_sb)

            # ---------- fc + outputs ----------
            for qt in range(QT):
                pf = ps1.tile([128, 512], F32, tag="p1")
                for h in range(NH):
                    nc.tensor.matmul(
                        pf,
                        OT[:, h, qt * 128:(qt + 1) * 128],
                        wf_sb[:, h, :],
                        start=(h == 0), stop=(h == NH - 1))
                o_sb = stage.tile([128, 512], F32, tag="osb")
                nc.scalar.copy(o_sb, pf)
                nc.sync.dma_start(out=out_part[qt * 128:(qt + 1) * 128, :], in_=o_sb)
                nc.sync.dma_start(out=attn_part[qt * 128:(qt + 1) * 128, :],
                                  in_=attn_acc[:, qt, :])

    nc.compile()
    return nc


def _get_nc():
    global _NC_CACHE
    if _NC_CACHE is None:
        _NC_CACHE = build_nc()
    return _NC_CACHE


def make_in_maps(q, k, v, Wq, bq, Wk, bk, Wv, bv, Wf):
    q = np.asarray(q); k = np.asarray(k); v = np.asarray(v)
    Wq = np.asarray(Wq); Wk = np.asarray(Wk); Wv = np.asarray(Wv)
    Wf = np.asarray(Wf)
    bq = np.asarray(bq, dtype=np.float32)
    bk = np.asarray(bk, dtype=np.float32)
    bv = np.asarray(bv, dtype=np.float32)
    in_maps = []
    for c in range(N_CORES):
        b, g = c // 4, c % 4
        cs = slice(g * DP, (g + 1) * DP)          # projection columns
        in_maps.append({
            "xqT": np.ascontiguousarray(q[b].T.astype(np.float16)),
            "xkT": np.ascontiguousarray(k[b].T.astype(np.float16)),
            "xvT": np.ascontiguousarray(v[b].T.astype(np.float16)),
            "wq": np.ascontiguousarray(Wq[:, cs].astype(np.float16)),
            "wk": np.ascontiguousarray(Wk[:, cs].astype(np.float16)),
            "wv": np.ascontiguousarray(Wv[:, cs].astype(np.float16)),
            "wf": np.ascontiguousarray(
                Wf[cs, :].astype(np.float16).reshape(NH, SZ, D)),
            "bq": np.ascontiguousarray(bq[cs]),
            "bk": np.ascontiguousarray(bk[cs]),
            "bv": np.ascontiguousarray(bv[cs].reshape(1, DP)),
        })
    return in_maps


def kernel(q, k, v, mask, Wq, bq, Wk, bk, Wv, bv, Wf, bf):
    bf = np.asarray(bf, dtype=np.float32)
    nc = _get_nc()
    in_maps = make_in_maps(q, k, v, Wq, bq, Wk, bk, Wv, bv, Wf)
    res = run_bass_kernel_spmd(nc, in_maps, list(range(N_CORES)))

    out = np.zeros((B, S, D), np.float32)
    attn_sum = np.zeros((B, S, S), np.float32)
    for c in range(N_CORES):
        b = c // 4
        out[b] += res.results[c]["out_part"]
        attn_sum[b] += res.results[c]["attn_part"].astype(np.float32)
    out += bf[None, None, :]
    return out, attn_sum


# revision 13
# speedup vs baseline: 12619.7714x; 15.5607x over previous
"""MultiHeadAttention TRN2 Bass kernel (nn_MultiHeadAttention_51565377356100).

Full inputs in, full outputs out. Sharding: 8 cores = 2 batches x 4 head-groups
(16 of the 64 effective heads each). Each core computes, for its (batch b,
heads hg*16..hg*16+15):
  qhT/khT = (x @ W{q,k} + b)^T per head      [fp16, via f32r-free fp16 matmuls]
  vh'     = x @ Wv + b, with a ones-column appended (rowsum rides attn@V)
  S       = qh @ kh^T (raw dot, scale 1/8 fused into exp)
  attnU   = exp(S/8)  [ACT, fp16, fp32 accum_out -> rowsum]
  attn_acc += attnU * (1/rowsum)   [fused scalar_tensor_tensor, fp16]
  attnUT  = DMA-xbar transpose of attnU (fp16)
  O'      = [vh | 1]^T @ attnUT    -> row 64 = fp16-consistent rowsum
  OT      = O'[0:64] * broadcast(1/rowsum)   [K=1 ones matmul + DVE mul]
  out_part = sum_h OT_h^T @ Wf_h   [PSUM-accumulated over the 16 heads]
Host: out = sum over 4 head-group cores + bf;  attn_sum likewise (then the
mask is all-False by construction, so it is a no-op).
"""
import numpy as np

import concourse.bass as bass
import concourse.tile as tile
from concourse import bacc, mybir
from concourse.bass_utils import run_bass_kernel_spmd

F32 = mybir.dt.float32
F32R = mybir.dt.float32r
F16 = mybir.dt.float16
AF = mybir.ActivationFunctionType
ALU = mybir.AluOpType

B, S, D = 2, 1024, 512
NH = 16          # heads per core (of 64 total)
SZ = 64          # head dim
DP = NH * SZ     # per-core projection width = 1024
N_CORES = 8

_NC_CACHE = None
import contextlib

def _nullcm():
    return contextlib.nullcontext()


def build_nc(loop_n=1, variant='full'):
    skips = set(variant.split(',')) - {'full'}
    nc = bacc.Bacc("TRN2", target_bir_lowering=False, debug=False,
                   num_devices=N_CORES)

    # ---- DRAM I/O (per-core shapes) ----
    xqT = nc.dram_tensor("xqT", [D, S], F16, kind="ExternalInput").ap()
    xkT = nc.dram_tensor("xkT", [D, S], F16, kind="ExternalInput").ap()
    xvT = nc.dram_tensor("xvT", [D, S], F16, kind="ExternalInput").ap()
    wq = nc.dram_tensor("wq", [D, DP], F16, kind="ExternalInput").ap()
    wk = nc.dram_tensor("wk", [D, DP], F16, kind="ExternalInput").ap()
    wv = nc.dram_tensor("wv", [D, DP], F16, kind="ExternalInput").ap()
    wf = nc.dram_tensor("wf", [NH, SZ, D], F16, kind="ExternalInput").ap()
    bq = nc.dram_tensor("bq", [DP], F32, kind="ExternalInput").ap()
    bk = nc.dram_tensor("bk", [DP], F32, kind="ExternalInput").ap()
    bv = nc.dram_tensor("bv", [1, DP], F32R, kind="ExternalInput").ap()

    out_part = nc.dram_tensor("out_part", [S, D], F32, kind="ExternalOutput").ap()
    attn_part = nc.dram_tensor("attn_part", [S, S], F16, kind="ExternalOutput").ap()

    KT = D // 128            # 4 contraction tiles for projections
    MT = DP // 128           # 8 outdim tiles (2 heads each)
    QT = S // 128            # 8 token tiles
    KB = S // 128            # 8 key tiles

    with tile.TileContext(nc) as tc:
        with (
            tc.tile_pool(name="fix", bufs=1) as fix,          # persistent tensors
            tc.tile_pool(name="stream", bufs=3) as stream,    # attnU tiles
            tc.tile_pool(name="tp", bufs=2) as tp,            # attnUT double-buffer
            tc.tile_pool(name="sm", bufs=2) as sm,            # small stats tiles
            tc.tile_pool(name="stage", bufs=2) as stage,      # psum->sbuf staging
            tc.tile_pool(name="ps1", bufs=2, space="PSUM") as ps1,   # proj/fc psum
            tc.tile_pool(name="pso", bufs=2, space="PSUM") as pso,   # O' psum
            tc.tile_pool(name="ps2", bufs=2, space="PSUM") as ps2,   # scores psum
        ):
          with (tc.For_i(0, loop_n, 1) if loop_n > 1 else _nullcm()):
            # ---------- load inputs ----------
            xq_sb = tp.tile([128, KT, S], F16, tag="attnUT")
            xk_sb = tp.tile([128, KT, S], F16, tag="attnUT")
            xv_sb = tp.tile([128, KT, S], F16, tag="attnUT")
            for kt in range(KT):
                nc.sync.dma_start(out=xq_sb[:, kt, :], in_=xqT[kt * 128:(kt + 1) * 128, :])
                nc.sync.dma_start(out=xk_sb[:, kt, :], in_=xkT[kt * 128:(kt + 1) * 128, :])
                nc.sync.dma_start(out=xv_sb[:, kt, :], in_=xvT[kt * 128:(kt + 1) * 128, :])
            wq_sb = stage.tile([128, KT, DP], F16, tag="W")
            wk_sb = stage.tile([128, KT, DP], F16, tag="W")
            wv_sb = stage.tile([128, KT, DP], F16, tag="W")
            for kt in range(KT):
                nc.sync.dma_start(out=wq_sb[:, kt, :], in_=wq[kt * 128:(kt + 1) * 128, :])
                nc.sync.dma_start(out=wk_sb[:, kt, :], in_=wk[kt * 128:(kt + 1) * 128, :])
                nc.sync.dma_start(out=wv_sb[:, kt, :], in_=wv[kt * 128:(kt + 1) * 128, :])
            wf_sb = fix.tile([64, NH, D], F16, tag="wf")
            for h in range(NH):
                nc.sync.dma_start(out=wf_sb[:, h, :], in_=wf[h, :, :])
            bq_sb = fix.tile([128, MT], F32, tag="bq")
            bk_sb = fix.tile([128, MT], F32, tag="bk")
            nc.sync.dma_start(out=bq_sb, in_=bq.rearrange("(m p) -> p m", p=128))
            nc.sync.dma_start(out=bk_sb, in_=bk.rearrange("(m p) -> p m", p=128))
            bv_row = fix.tile([1, DP], F32R, tag="bvrow")
            nc.sync.dma_start(out=bv_row, in_=bv)

            # ones rows (f32r) for K=1 broadcast matmuls; row 64 used for rb.
            ones_f = fix.tile([128, 128], F32, tag="onesf")
            nc.vector.memset(ones_f, 1.0)
            ones_sb = fix.tile([128, 128], F32R, tag="ones")
            nc.vector.tensor_copy(ones_sb, ones_f)

            # bv broadcast [128, DP] fp32 via K=1 matmul
            bv_bc = fix.tile([128, DP], F16, tag="bvbc")
            for n5 in range(DP // 512):
                pbc = ps1.tile([128, 512], F32, tag="p1")
                nc.tensor.matmul(pbc, ones_sb[0:1, :], bv_row[:, n5 * 512:(n5 + 1) * 512],
                                 start=True, stop=True)
                nc.vector.tensor_copy(bv_bc[:, n5 * 512:(n5 + 1) * 512], pbc)

            # ---------- projections ----------
            qhT_sb = fix.tile([128, MT, S], F16, tag="qhT")
            khT_sb = fix.tile([128, MT, S], F16, tag="khT")
            for (w_sb, x_sb, b_sb, o_sb) in ((wq_sb, xq_sb, bq_sb, qhT_sb),
                                             (wk_sb, xk_sb, bk_sb, khT_sb)):
                for m in range(MT):
                    for n in range(S // 512):
                        pp = ps1.tile([128, 512], F32, tag="p1")
                        for kt in range(KT):
                            nc.tensor.matmul(
                                pp,
                                w_sb[:, kt, m * 128:(m + 1) * 128],
                                x_sb[:, kt, n * 512:(n + 1) * 512],
                                start=(kt == 0), stop=(kt == KT - 1))
                        nc.scalar.activation(
                            out=o_sb[:, m, n * 512:(n + 1) * 512], in_=pp,
                            func=AF.Identity, bias=b_sb[:, m:m + 1], scale=1.0)

            # vh [128, tt, NH, 64] fp16
            vhp = fix.tile([128, QT, NH, SZ], F16, tag="vhp")
            for tt in range(QT):
                for n5 in range(DP // 512):
                    pv = ps1.tile([128, 512], F32, tag="p1")
                    for kt in range(KT):
                        nc.tensor.matmul(
                            pv,
                            xv_sb[:, kt, tt * 128:(tt + 1) * 128],
                            wv_sb[:, kt, n5 * 512:(n5 + 1) * 512],
                            start=(kt == 0), stop=(kt == KT - 1))
                    nc.vector.tensor_add(
                        vhp[:, tt, n5 * 8:(n5 + 1) * 8, :],
                        pv.rearrange("p (h d) -> p h d", d=SZ),
                        bv_bc[:, n5 * 512:(n5 + 1) * 512].rearrange(
                            "p (h d) -> p h d", d=SZ))

            # ---------- attention ----------
            attn_acc = fix.tile([128, QT, S], F16, tag="acc")
            out_acc_written = False
            OT = fix.tile([64, NH, S], F16, tag="OT")
            for h in range(NH):
                m, po = h // 2, (h % 2) * 64
                attnUT = tp.tile([128, KB, S], F16, tag="attnUT")
                for qt in range(QT):
                    attnU = stream.tile([128, S], F16, tag="attnU")
                    if 'sexp' in skips:
                        nc.vector.memset(attnU[:, 0:1], 0.0)
                    if 'stt' in skips or 'sexp' in skips:
                        if h == 0:
                            nc.vector.memset(attn_acc[:, qt, 0:1], 0.0)
                    if 'sexp' not in skips:
                        sp = ps2.tile([128, S], F32, tag="p2")
                        if 's' not in skips:
                            for k5 in range(S // 512):
                                nc.tensor.matmul(
                                    sp[:, k5 * 512:(k5 + 1) * 512],
                                    qhT_sb[po:po + 64, m, qt * 128:(qt + 1) * 128],
                                    khT_sb[po:po + 64, m, k5 * 512:(k5 + 1) * 512],
                                    start=True, stop=True)
                        rs = sm.tile([128, 1], F32, tag="rs")
                        nc.scalar.activation(out=attnU, in_=sp, func=AF.Exp,
                                             scale=0.125, accum_out=rs)
                        r = sm.tile([128, 1], F32, tag="r")
                        nc.vector.reciprocal(r, rs)
                        attn_n = stream.tile([128, S], F16, tag="attnN")
                        nc.vector.tensor_scalar_mul(attn_n, attnU, r)
                        if 'stt' not in skips:
                            if h == 0:
                                nc.vector.tensor_copy(attn_acc[:, qt, :], attn_n)
                            else:
                                nc.vector.tensor_add(attn_acc[:, qt, :], attn_n,
                                                     attn_acc[:, qt, :])
                    if 'tr' not in skips and 'sexp' not in skips:
                        nc.sync.dma_start_transpose(
                            out=attnUT[:, :, qt * 128:(qt + 1) * 128], in_=attn_n)
                    elif qt == 0:
                        nc.vector.memset(attnUT[:, 0, 0:1], 0.0)
                if 'o' in skips:
                    nc.vector.memset(OT[:, h, 0:1], 0.0)
                for q5 in range(0 if 'o' in skips else S // 512):
                    po_ = pso.tile([64, 512], F32, tag="po")
                    for kb in range(KB):
                        nc.tensor.matmul(
                            po_,
                            vhp[:, kb, h, :],
                            attnUT[:, kb, q5 * 512:(q5 + 1) * 512],
                            start=(kb == 0), stop=(kb == KB - 1))
                    if 'ocp' in skips:
                        nc.vector.tensor_copy(OT[:, h, q5 * 512:q5 * 512 + 1], po_[:, 0:1])
                    else:
                        nc.vector.tensor_copy(OT[:, h, q5 * 512:(q5 + 1) * 512], po_)

            # ---------- fc + outputs ----------
            for qt in range(QT):
                o_sb = stage.tile([128, 512], F32, tag="osb")
                if 'fc' in skips:
                    nc.vector.memset(o_sb[:, 0:1], 0.0)
                if 'fc' not in skips:
                    pf = ps1.tile([128, 512], F32, tag="p1")
                    for h in range(NH):
                        nc.tensor.matmul(
                            pf,
                            OT[:, h, qt * 128:(qt + 1) * 128],
                            wf_sb[:, h, :],
                            start=(h == 0), stop=(h == NH - 1))
                    nc.scalar.copy(o_sb, pf)
                nc.sync.dma_start(out=out_part[qt * 128:(qt + 1) * 128, :], in_=o_sb)
                nc.sync.dma_start(out=attn_part[qt * 128:(qt + 1) * 128, :],
                                  in_=attn_acc[:, qt, :])

    nc.compile()
    return nc


def _get_nc():
    global _NC_CACHE
    if _NC_CACHE is None:
        _NC_CACHE = build_nc()
    return _NC_CACHE


def make_in_maps(q, k, v, Wq, bq, Wk, bk, Wv, bv, Wf):
    q = np.asarray(q); k = np.asarray(k); v = np.asarray(v)
    Wq = np.asarray(Wq); Wk = np.asarray(Wk); Wv = np.asarray(Wv)
    Wf = np.asarray(Wf)
    bq = np.asarray(bq, dtype=np.float32)
    bk = np.asarray(bk, dtype=np.float32)
    bv = np.asarray(bv, dtype=np.float32)
    in_maps = []
    for c in range(N_CORES):
        b, g = c // 4, c % 4
        cs = slice(g * DP, (g + 1) * DP)          # projection columns
        in_maps.append({
            "xqT": np.ascontiguousarray(q[b].T.astype(np.float16)),
            "xkT": np.ascontiguousarray(k[b].T.astype(np.float16)),
            "xvT": np.ascontiguousarray(v[b].T.astype(np.float16)),
            "wq": np.ascontiguousarray(Wq[:, cs].astype(np.float16)),
            "wk": np.ascontiguousarray(Wk[:, cs].astype(np.float16)),
            "wv": np.ascontiguousarray(Wv[:, cs].astype(np.float16)),
            "wf": np.ascontiguousarray(
                Wf[cs, :].astype(np.float16).reshape(NH, SZ, D)),
            "bq": np.ascontiguousarray(bq[cs]),
            "bk": np.ascontiguousarray(bk[cs]),
            "bv": np.ascontiguousarray(bv[cs].reshape(1, DP)),
        })
    return in_maps


def kernel(q, k, v, mask, Wq, bq, Wk, bk, Wv, bv, Wf, bf):
    bf = np.asarray(bf, dtype=np.float32)
    nc = _get_nc()
    in_maps = make_in_maps(q, k, v, Wq, bq, Wk, bk, Wv, bv, Wf)
    res = run_bass_kernel_spmd(nc, in_maps, list(range(N_CORES)))

    out = np.zeros((B, S, D), np.float32)
    attn_sum = np.zeros((B, S, S), np.float32)
    for c in range(N_CORES):
        b = c // 4
        out[b] += res.results[c]["out_part"]
        attn_sum[b] += res.results[c]["attn_part"].astype(np.float32)
    out += bf[None, None, :]
    return out, attn_sum
